# revision 1
# baseline (speedup 1.0000x reference)
"""Two-branch attention kernel for Trainium2 (8 NeuronCores, batch-parallel).

out1 = proj(softmax(q k^T / 8) v),  out2 = proj(softmax(q k2^T / 8) v2)
with q,k,v from x and k2,v2 from x2 (q shared across branches).

Sharding: batch dim (8) -> one batch element per core. No collectives.

Layout strategy (per core, transpose-free attention):
  host passes x^T, x2^T, W_qkv^T, W_proj^T, broadcast bias.
  QKV:  qT,kT [dim,tok] via W-stationary matmuls; v [tok,dim] via
        x-stationary matmuls, written into a ones-augmented buffer.
        k2T/v2 spilled to DRAM to bound SBUF.
  Attn: S^T tile = kT(stationary) @ qT(moving); exp on ScalarE (scale=1/8,
        no max subtraction -- scores are ~N(0,1), exp range is safe) writes
        P^T straight to SBUF; AV accumulates o_aug^T = [v|1]^T @ P^T giving
        both o^T and the softmax denominators r; normalize o^T by 1/r with a
        partition-broadcast multiply.
  Proj: o^T tiles stationary, stream W_proj^T, add bias, DMA out.
"""
import sys
for _p in ('/opt/trn_rl_repo',):
    if _p not in sys.path:
        sys.path.insert(0, _p)

import numpy as np

# ----------------------------------------------------------------------------
# knobs
MODE = 'f32r+bf16p'  # informational only
OT64 = False         # True: o^T stored [64,12,1024] (no partition-offset
                     # writes, proj uses 12 K=64 tiles); False: [128,6,1024]
BCAST_DMA = False     # broadcast 1/r via DMA (else gpsimd partition_broadcast)

EXP_COMBINED = True
ACT_EVICT = True
STRUCT = 2           # 0: separate S psums + AV after exp + no proj interleave
                     # 1: shared S tile + AV interleaved
                     # 2: + proj thunks interleaved

B, N, D, H, HD = 8, 1024, 768, 12, 64
SCALE = HD ** -0.5
NDT = D // 128       # 6 dim tiles
NQT = N // 128       # 8 token tiles
P = 128

# ----------------------------------------------------------------------------
# workaround: walrus rejects >2 sem waits on one instruction; TileContext's
# tail drain carries one wait per active logical proc. Split them across
# single-wait SP nops and emit a bare drain.
def _install_tilefix():
    import bass_rust
    import concourse.tile as tile

    def _drain_and_barrier_split(self, tick_clock, wait_clock):
        gc = tick_clock.global_clock
        ticks = [gc[i] for i in range(27)]
        for i, t in enumerate(ticks):
            if t > 0:
                vc = bass_rust.VectorClock(
                    [t if j == i else 0 for j in range(len(ticks))])
                nop = self.nc.sync.nop()
                wait_clock.add_sem_waits(
                    nop.ins, bass_rust.ScopedClock({None: vc}))
        self.nc.sync.drain()
        self.nc.all_engine_barrier()
        assert self.sems is not None
        popped = self.nc._tile_sem_poison_stack.pop()
        assert popped is self._sem_poison
        self.nc.clear_and_free_semaphores(list(self.sems.allocated().values()))
        self.nc.all_engine_barrier()

    tile.TileContext._drain_and_barrier = _drain_and_barrier_split


def _split_multiwaits(nc, max_waits=1):
    """walrus codegen rejects instructions carrying more than `max_waits`
    sync waits; hoist the extras onto same-engine nops placed just before."""
    import bass_rust
    import concourse.mybir as mybir
    cnt = 0
    for bb in nc.main_func.blocks:
        insts = bb.instructions
        i = 0
        while i < len(insts):
            ins = insts[i]
            si = getattr(ins, 'sync_info', None)
            if si is not None and si.on_wait and len(si.on_wait) > max_waits:
                waits = list(si.on_wait)
                extras, keep = waits[:-max_waits], waits[-max_waits:]
                for w in extras:
                    nop = mybir.InstNoOp(name=f"I-swx{cnt}", ins=[], outs=[])
                    cnt += 1
                    nop.engine = ins.engine
                    nop.sync_info = bass_rust.SyncInfo(on_wait=[w],
                                                       on_update=[])
                    insts.insert(i, nop)
                    i += 1
                ins.sync_info = bass_rust.SyncInfo(
                    on_wait=keep, on_update=list(si.on_update))
            i += 1
    return cnt


_built = None


def _build():
    """Build the SPMD bass program once. Returns (nc, n_split_waits)."""
    global _built
    if _built is not None:
        return _built
    _install_tilefix()
    from contextlib import ExitStack
    import concourse.bass as bass
    import concourse.tile as tile
    from concourse import mybir

    dt = mybir.dt
    ddt = dt.float32r          # matmul dtype for QKV / QK / proj operands
    pdt = dt.bfloat16          # attention probabilities P^T and V storage

    nc = bass.Bass("TRN2", target_bir_lowering=False, debug=False,
                   num_devices=8)

    # DRAM I/O (per core)
    xt_d = nc.dram_tensor("xt", [D, N], ddt, kind="ExternalInput")
    x2t_d = nc.dram_tensor("x2t", [D, N], ddt, kind="ExternalInput")
    wqk_d = nc.dram_tensor("wqk", [D, 2 * D], ddt, kind="ExternalInput")
    wv_d = nc.dram_tensor("wv", [D, D], ddt, kind="ExternalInput")
    wp_d = nc.dram_tensor("wp", [D, D], ddt, kind="ExternalInput")
    bias_d = nc.dram_tensor("bias", [P, D], dt.float32, kind="ExternalInput")
    ones_d = nc.dram_tensor("ones", [P, H, 1], pdt, kind="ExternalInput")
    out_d = nc.dram_tensor("out", [2, N, D], dt.float32,
                           kind="ExternalOutput")

    AUG = HD + 1  # 65: head dim + ones column for row sums

    with tile.TileContext(nc) as tc, ExitStack() as top:
        pp_s = top.enter_context(tc.tile_pool(name="ps_s", bufs=1,
                                              space="PSUM"))
        pp_o = top.enter_context(tc.tile_pool(name="ps_o", bufs=2,
                                              space="PSUM"))
        dram = top.enter_context(tc.tile_pool(name="dram", bufs=1,
                                              space="DRAM"))
        dram_rb = top.enter_context(tc.tile_pool(name="dram_rb", bufs=2,
                                                 space="DRAM"))
        persist = top.enter_context(tc.tile_pool(name="persist", bufs=1))
        pool_kv = top.enter_context(tc.tile_pool(name="kv", bufs=1))

        qT = persist.tile([P, NDT, N], ddt, tag="qT")
        wp_t = persist.tile([P, NDT, D], ddt, tag="wp")
        nc.sync.dma_start(
            out=wp_t, in_=wp_d[:].rearrange("(g p) d -> p g d", p=P))
        bias_t = persist.tile([P, D], dt.float32, tag="bias")
        nc.sync.dma_start(out=bias_t, in_=bias_d[:])

        k2_spill = dram.tile([NDT, P, N], ddt, tag="k2s")
        v2_spill = dram.tile([NQT, P, H, HD], pdt, tag="v2s")

        # ================= phase A: QKV (x then x2-with-spill) ==========
        kT = pool_kv.tile([P, NDT, N], ddt, tag="kT")
        vaug = pool_kv.tile([P, NQT, H * AUG], pdt, tag="vaug")

        def load_ones(vaug_t):
            for t in range(NQT):
                nc.sync.dma_start(
                    out=vaug_t[:, t, :].rearrange("p (h e) -> p h e",
                                                  e=AUG)[:, :, HD:AUG],
                    in_=ones_d[:])

        with tc.tile_pool(name="xa", bufs=2) as pool_x, \
             tc.tile_pool(name="wst", bufs=10) as pool_wst, \
             tc.tile_pool(name="wvp", bufs=1) as pool_wv, \
             tc.tile_pool(name="ev", bufs=3) as pool_ev:

            def qkv_T_form(xt_t, wcol0, dst_sb, dst_dram):
                """out-tiles [128, N] accumulated over in-tiles; PSUM evict
                on ScalarE (ACT idle in this phase)."""
                for o in range(NDT):
                    psf = pp_s.tile([P, 2, N] if STRUCT >= 1 else [P, N],
                                    dt.float32, tag="S")
                    ps = psf.rearrange("p (a n) -> p a n", a=1) \
                        if STRUCT < 1 else psf
                    for i in range(NDT):
                        wt = pool_wst.tile([P, P], ddt, tag="wt")
                        nc.sync.dma_start(
                            out=wt,
                            in_=wqk_d[i * P:(i + 1) * P,
                                      wcol0 + o * P: wcol0 + (o + 1) * P])
                        for c in range(2):
                            nc.tensor.matmul(
                                ps[:, 0, c * 512:(c + 1) * 512],
                                wt[:],
                                xt_t[:, i, c * 512:(c + 1) * 512],
                                start=(i == 0), stop=(i == NDT - 1))
                    cp = nc.scalar.copy if ACT_EVICT else nc.vector.tensor_copy
                    if dst_sb is not None:
                        cp(dst_sb[:, o, :], ps[:, 0, :])
                    else:
                        ev = pool_ev.tile([P, N], ddt, tag="ev")
                        cp(ev[:], ps[:, 0, :])
                        nc.sync.dma_start(out=dst_dram[o], in_=ev[:])

            def v_form(xt_t, wv_t, vaug_t, dst_dram):
                for t in range(NQT):
                    psf = pp_s.tile([P, 2, N] if STRUCT >= 1 else [P, N],
                                    dt.float32, tag="S")
                    ps = psf.rearrange("p (a n) -> p a n", a=1) \
                        if STRUCT < 1 else psf
                    for i in range(NDT):
                        for c0, cn in ((0, 512), (512, 256)):
                            nc.tensor.matmul(
                                ps[:, 0, c0:c0 + cn],
                                xt_t[:, i, t * P:(t + 1) * P],
                                wv_t[:, i, c0:c0 + cn],
                                start=(i == 0), stop=(i == NDT - 1))
                    src = ps[:, 0, 0:D].rearrange("p (h e) -> p h e", e=HD)
                    cp = nc.scalar.copy if ACT_EVICT else nc.vector.tensor_copy
                    if vaug_t is not None:
                        dstv = vaug_t[:, t, :].rearrange(
                            "p (h e) -> p h e", e=AUG)[:, :, 0:HD]
                        cp(dstv, src)
                    else:
                        ev = pool_ev.tile([P, H, HD], pdt, tag="evv")
                        cp(ev[:], src)
                        nc.sync.dma_start(out=dst_dram[t], in_=ev[:])

            xt_t = pool_x.tile([P, NDT, N], ddt, tag="xt")
            nc.sync.dma_start(out=xt_t,
                              in_=xt_d[:].rearrange("(i p) n -> p i n", p=P))
            wv_t = pool_wv.tile([P, NDT, D], ddt, tag="wv")
            nc.sync.dma_start(out=wv_t,
                              in_=wv_d[:].rearrange("(i p) d -> p i d", p=P))

            qkv_T_form(xt_t, 0, qT, None)            # qT
            qkv_T_form(xt_t, D, kT, None)            # kT
            load_ones(vaug)
            v_form(xt_t, wv_t, vaug, None)           # v -> vaug

            x2t_t = pool_x.tile([P, NDT, N], ddt, tag="xt")
            nc.sync.dma_start(out=x2t_t,
                              in_=x2t_d[:].rearrange("(i p) n -> p i n", p=P))
            qkv_T_form(x2t_t, D, None, k2_spill)     # k2T -> dram
            v_form(x2t_t, wv_t, None, v2_spill)      # v2 -> dram

        # ================= phase B: attention + proj ====================
        pool_pt = top.enter_context(tc.tile_pool(name="pt", bufs=1))
        pool_ot = top.enter_context(tc.tile_pool(name="ot", bufs=2))
        pool_res = top.enter_context(tc.tile_pool(name="res", bufs=2))
        pool_sm = top.enter_context(tc.tile_pool(name="sm", bufs=1))
        pool_osb = top.enter_context(tc.tile_pool(name="osb", bufs=2))

        def proj_qi(ot_t, br, qi):
            ps = pp_o.tile([P, D], dt.float32, tag="O")
            for g in range(NDT):
                for c0, cn in ((0, 512), (512, 256)):
                    nc.tensor.matmul(
                        ps[:, c0:c0 + cn],
                        ot_t[:, g, qi * P:(qi + 1) * P],
                        wp_t[:, g, c0:c0 + cn],
                        start=(g == 0), stop=(g == NDT - 1))
            res = pool_res.tile([P, D], dt.float32, tag="res")
            nc.vector.tensor_add(res[:], ps[:], bias_t[:])
            nc.sync.dma_start(out=out_d[br, qi * P:(qi + 1) * P, :],
                              in_=res[:])

        def attention(kT_t, vaug_t, br, extra):
            """head-pair ladder; STRUCT selects aggressiveness."""
            ot = pool_ot.tile([P, NDT, N], ddt, tag="ot")
            for g in range(NDT):
                pt2 = pool_pt.tile([P, 2, NQT, N], pdt, tag="pt")
                po = [pp_o.tile([AUG, N], dt.float32, tag="O",
                                name=f"po{br}_{g}_{hh}")
                      for hh in range(2)]

                def emit_av(kj):
                    for hh in range(2):
                        h = 2 * g + hh
                        for c in range(2):
                            nc.tensor.matmul(
                                po[hh][:, c * 512:(c + 1) * 512],
                                vaug_t[:, kj, h * AUG:(h + 1) * AUG],
                                pt2[:, hh, kj, c * 512:(c + 1) * 512],
                                start=(kj == 0), stop=(kj == NQT - 1),
                                skip_group_check=(STRUCT >= 1))

                for kj in range(NQT):
                    if STRUCT >= 1:
                        ps = pp_s.tile([P, 2, N], dt.float32, tag="S")
                        pse, pso = ps[:, 0, :], ps[:, 1, :]
                    else:
                        pse = pp_s.tile([P, N], dt.float32, tag="S")
                        pso = pp_s.tile([P, N], dt.float32, tag="S2")
                    for c in range(2):
                        nc.tensor.matmul(
                            pse[:, c * 512:(c + 1) * 512],
                            kT_t[0:HD, g, kj * P:(kj + 1) * P],
                            qT[0:HD, g, c * 512:(c + 1) * 512],
                            start=True, stop=True)
                        nc.tensor.matmul(
                            pso[:, c * 512:(c + 1) * 512],
                            kT_t[HD:P, g, kj * P:(kj + 1) * P],
                            qT[HD:P, g, c * 512:(c + 1) * 512],
                            start=True, stop=True)
                    if EXP_COMBINED and STRUCT >= 1:
                        nc.scalar.activation(
                            pt2[:, :, kj, :], ps[:],
                            mybir.ActivationFunctionType.Exp, scale=SCALE)
                    else:
                        nc.scalar.activation(
                            pt2[:, 0, kj, :], pse,
                            mybir.ActivationFunctionType.Exp, scale=SCALE)
                        nc.scalar.activation(
                            pt2[:, 1, kj, :], pso,
                            mybir.ActivationFunctionType.Exp, scale=SCALE)
                    if STRUCT >= 1 and kj % 4 == 3:
                        for kjb in range(kj - 3, kj + 1):
                            emit_av(kjb)
                if STRUCT == 0:
                    for kj in range(NQT):
                        emit_av(kj)
                for hh in range(2):
                    row = hh * HD
                    osb = pool_osb.tile([AUG, N], dt.float32, tag="osb")
                    nc.vector.tensor_copy(osb[:], po[hh][:])
                    r_t = pool_sm.tile([1, N], dt.float32, tag="r")
                    nc.vector.reciprocal(r_t[:], osb[HD:HD + 1, :])
                    r_bounce = dram_rb.tile([1, N], dt.float32, tag="rbb")
                    nc.sync.dma_start(out=r_bounce[:], in_=r_t[:])
                    rb_t = pool_sm.tile([HD, N], dt.float32, tag="rb")
                    nc.sync.dma_start(
                        out=rb_t[:],
                        in_=r_bounce[0, :].partition_broadcast(HD))
                    nc.vector.tensor_mul(
                        ot[row:row + HD, g, :], osb[0:HD, :], rb_t[:])
                if STRUCT >= 2 and extra:
                    extra.pop(0)()
                    if len(extra) > NDT - g - 1:
                        extra.pop(0)()
            while extra:
                extra.pop(0)()
            return ot

        ot0 = attention(kT, vaug, 0, [])

        # branch 2 k/v reload, then attention with proj(br0) interleaved
        kT2 = pool_kv.tile([P, NDT, N], ddt, tag="kT")
        nc.sync.dma_start(out=kT2,
                          in_=k2_spill[:].rearrange("g p n -> p g n"))
        vaug2 = pool_kv.tile([P, NQT, H * AUG], pdt, tag="vaug")
        load_ones(vaug2)
        for t in range(NQT):
            nc.sync.dma_start(
                out=vaug2[:, t, :].rearrange("p (h e) -> p h e",
                                             e=AUG)[:, :, 0:HD],
                in_=v2_spill[t])

        extra = ([(lambda qi=qi: proj_qi(ot0, 0, qi)) for qi in range(NQT)]
                 if STRUCT >= 2 else [])
        ot1 = attention(kT2, vaug2, 1, extra)
        if STRUCT < 2:
            for qi in range(NQT):
                proj_qi(ot0, 0, qi)
        for qi in range(NQT):
            proj_qi(ot1, 1, qi)

    n = _split_multiwaits(nc)
    _built = (nc, n)
    return _built


def _host_prep(x, x2, qkv_w, proj_w, proj_b):
    """-> list of 8 per-core input maps. Matmul operands are float32
    (device treats them as float32r); P/V-side constants are bfloat16."""
    import ml_dtypes
    f32 = lambda a: np.ascontiguousarray(a, dtype=np.float32)

    xt = np.ascontiguousarray(np.transpose(np.asarray(x), (0, 2, 1)))
    x2t = np.ascontiguousarray(np.transpose(np.asarray(x2), (0, 2, 1)))
    wqk = f32(np.asarray(qkv_w)[:2 * D].T)      # [768, 1536]
    wv = f32(np.asarray(qkv_w)[2 * D:].T)       # [768, 768]
    wp = f32(np.asarray(proj_w).T)              # [768, 768]
    bias = np.broadcast_to(np.asarray(proj_b, dtype=np.float32),
                           (P, D)).copy()
    ones = np.ones((P, H, 1), dtype=ml_dtypes.bfloat16)
    maps = []
    for c in range(B):
        maps.append({
            "xt": f32(xt[c]), "x2t": f32(x2t[c]),
            "wqk": wqk, "wv": wv, "wp": wp, "bias": bias,
            "ones": ones,
        })
    return maps


def kernel(x, x2, qkv_w, proj_w, proj_b, trace=False, tmpdir=None):
    nc, _ = _build()
    from concourse.bass_utils import run_bass_kernel_spmd
    in_maps = _host_prep(x, x2, qkv_w, proj_w, proj_b)
    res = run_bass_kernel_spmd(nc, in_maps, list(range(B)), trace=trace,
                               tmpdir=tmpdir)
    kernel.last_exec_time_ns = res.exec_time_ns
    out = np.stack([res.results[c]["out"] for c in range(B)])  # [B,2,N,D]
    out1 = np.ascontiguousarray(out[:, 0])
    out2 = np.ascontiguousarray(out[:, 1])
    return (out1, out2)


kernel.last_exec_time_ns = None



# revision 13
# speedup vs baseline: 1.1098x; 1.1098x over previous
"""Two-branch attention kernel for Trainium2 (8 NeuronCores, batch-parallel).

out1 = proj(softmax(q k^T / 8) v),  out2 = proj(softmax(q k2^T / 8) v2)
with q,k,v from x and k2,v2 from x2 (q shared across branches).

Sharding: batch dim (8) -> one batch element per core. No collectives.

Layout strategy (per core, transpose-free attention):
  host passes x^T, x2^T, W_qkv^T, W_proj^T, broadcast bias.
  QKV:  qT,kT [dim,tok] via W-stationary matmuls; v [tok,dim] via
        x-stationary matmuls, written into a ones-augmented buffer.
        k2T/v2 spilled to DRAM to bound SBUF.
  Attn: S^T tile = kT(stationary) @ qT(moving); exp on ScalarE (scale=1/8,
        no max subtraction -- scores are ~N(0,1), exp range is safe) writes
        P^T straight to SBUF; AV accumulates o_aug^T = [v|1]^T @ P^T giving
        both o^T and the softmax denominators r; normalize o^T by 1/r with a
        partition-broadcast multiply.
  Proj: o^T tiles stationary, stream W_proj^T, add bias, DMA out.
"""
import sys
for _p in ('/opt/trn_rl_repo',):
    if _p not in sys.path:
        sys.path.insert(0, _p)

import numpy as np

# ----------------------------------------------------------------------------
# knobs
MODE = 'f32r+bf16p+fastnorm'  # informational only

B, N, D, H, HD = 8, 1024, 768, 12, 64
SCALE = HD ** -0.5
NDT = D // 128       # 6 dim tiles
NQT = N // 128       # 8 token tiles
P = 128

# ----------------------------------------------------------------------------
# workaround: walrus rejects >2 sem waits on one instruction; TileContext's
# tail drain carries one wait per active logical proc. Split them across
# single-wait SP nops and emit a bare drain.
def _install_tilefix():
    import bass_rust
    import concourse.tile as tile

    def _drain_and_barrier_split(self, tick_clock, wait_clock):
        gc = tick_clock.global_clock
        ticks = [gc[i] for i in range(27)]
        for i, t in enumerate(ticks):
            if t > 0:
                vc = bass_rust.VectorClock(
                    [t if j == i else 0 for j in range(len(ticks))])
                nop = self.nc.sync.nop()
                wait_clock.add_sem_waits(
                    nop.ins, bass_rust.ScopedClock({None: vc}))
        self.nc.sync.drain()
        self.nc.all_engine_barrier()
        assert self.sems is not None
        popped = self.nc._tile_sem_poison_stack.pop()
        assert popped is self._sem_poison
        self.nc.clear_and_free_semaphores(list(self.sems.allocated().values()))
        self.nc.all_engine_barrier()

    tile.TileContext._drain_and_barrier = _drain_and_barrier_split


def _split_multiwaits(nc, max_waits=1):
    """walrus codegen rejects instructions carrying more than `max_waits`
    sync waits; hoist the extras onto same-engine nops placed just before."""
    import bass_rust
    import concourse.mybir as mybir
    cnt = 0
    for bb in nc.main_func.blocks:
        insts = bb.instructions
        i = 0
        while i < len(insts):
            ins = insts[i]
            si = getattr(ins, 'sync_info', None)
            if si is not None and si.on_wait and len(si.on_wait) > max_waits:
                waits = list(si.on_wait)
                extras, keep = waits[:-max_waits], waits[-max_waits:]
                for w in extras:
                    nop = mybir.InstNoOp(name=f"I-swx{cnt}", ins=[], outs=[])
                    cnt += 1
                    nop.engine = ins.engine
                    nop.sync_info = bass_rust.SyncInfo(on_wait=[w],
                                                       on_update=[])
                    insts.insert(i, nop)
                    i += 1
                ins.sync_info = bass_rust.SyncInfo(
                    on_wait=keep, on_update=list(si.on_update))
            i += 1
    return cnt


_built = None


def _build():
    """Build the SPMD bass program once. Returns (nc, n_split_waits)."""
    global _built
    if _built is not None:
        return _built
    _install_tilefix()
    from contextlib import ExitStack
    import concourse.bass as bass
    import concourse.tile as tile
    from concourse import mybir

    dt = mybir.dt
    ddt = dt.float32r          # matmul dtype for QKV / QK / proj operands
    pdt = dt.bfloat16          # attention probabilities P^T and V storage

    nc = bass.Bass("TRN2", target_bir_lowering=False, debug=False,
                   num_devices=8)

    # DRAM I/O (per core)
    xt_d = nc.dram_tensor("xt", [D, N], ddt, kind="ExternalInput")
    x2t_d = nc.dram_tensor("x2t", [D, N], ddt, kind="ExternalInput")
    wqk_d = nc.dram_tensor("wqk", [D, 2 * D], ddt, kind="ExternalInput")
    wv_d = nc.dram_tensor("wv", [D, D], ddt, kind="ExternalInput")
    wp_d = nc.dram_tensor("wp", [D, D], ddt, kind="ExternalInput")
    bias_d = nc.dram_tensor("bias", [P, D], dt.float32, kind="ExternalInput")
    ones_d = nc.dram_tensor("ones", [P, H, 1], pdt, kind="ExternalInput")
    out_d = nc.dram_tensor("out", [2, N, D], dt.float32,
                           kind="ExternalOutput")

    AUG = HD + 1  # 65: head dim + ones column for row sums

    with tile.TileContext(nc) as tc, ExitStack() as top:
        pp_s = top.enter_context(tc.tile_pool(name="ps_s", bufs=2,
                                              space="PSUM"))
        pp_o = top.enter_context(tc.tile_pool(name="ps_o", bufs=2,
                                              space="PSUM"))
        dram = top.enter_context(tc.tile_pool(name="dram", bufs=1,
                                              space="DRAM"))
        dram_rb = top.enter_context(tc.tile_pool(name="dram_rb", bufs=2,
                                                 space="DRAM"))
        persist = top.enter_context(tc.tile_pool(name="persist", bufs=1))
        pool_kv = top.enter_context(tc.tile_pool(name="kv", bufs=1))

        qT = persist.tile([P, NDT, N], ddt, tag="qT")
        wp_t = persist.tile([P, NDT, D], ddt, tag="wp")
        nc.sync.dma_start(
            out=wp_t, in_=wp_d[:].rearrange("(g p) d -> p g d", p=P))
        bias_t = persist.tile([P, D], dt.float32, tag="bias")
        nc.sync.dma_start(out=bias_t, in_=bias_d[:])

        k2_spill = dram.tile([NDT, P, N], ddt, tag="k2s")
        v2_spill = dram.tile([NQT, P, H, HD], pdt, tag="v2s")

        # ================= phase A: QKV (x then x2-with-spill) ==========
        kT = pool_kv.tile([P, NDT, N], ddt, tag="kT")
        vaug = pool_kv.tile([P, NQT, H * AUG], pdt, tag="vaug")

        def load_ones(vaug_t):
            for t in range(NQT):
                nc.sync.dma_start(
                    out=vaug_t[:, t, :].rearrange("p (h e) -> p h e",
                                                  e=AUG)[:, :, HD:AUG],
                    in_=ones_d[:])

        with tc.tile_pool(name="xa", bufs=2) as pool_x, \
             tc.tile_pool(name="wst", bufs=10) as pool_wst, \
             tc.tile_pool(name="wvp", bufs=1) as pool_wv, \
             tc.tile_pool(name="ev", bufs=3) as pool_ev:

            def qkv_T_form(xt_t, wcol0, dst_sb, dst_dram):
                """out-tiles [128, N] accumulated over in-tiles; PSUM evict
                on ScalarE (ACT idle in this phase)."""
                for o in range(NDT):
                    ps = pp_s.tile([P, N], dt.float32, tag="S")
                    for i in range(NDT):
                        wt = pool_wst.tile([P, P], ddt, tag="wt")
                        nc.sync.dma_start(
                            out=wt,
                            in_=wqk_d[i * P:(i + 1) * P,
                                      wcol0 + o * P: wcol0 + (o + 1) * P])
                        for c in range(2):
                            nc.tensor.matmul(
                                ps[:, c * 512:(c + 1) * 512],
                                wt[:],
                                xt_t[:, i, c * 512:(c + 1) * 512],
                                start=(i == 0), stop=(i == NDT - 1))
                    if dst_sb is not None:
                        nc.scalar.copy(dst_sb[:, o, :], ps[:])
                    else:
                        ev = pool_ev.tile([P, N], ddt, tag="ev")
                        nc.scalar.copy(ev[:], ps[:])
                        nc.sync.dma_start(out=dst_dram[o], in_=ev[:])

            def v_form(xt_t, wv_t, vaug_t, dst_dram):
                for t in range(NQT):
                    ps = pp_s.tile([P, N], dt.float32, tag="S")
                    for i in range(NDT):
                        for c0, cn in ((0, 512), (512, 256)):
                            nc.tensor.matmul(
                                ps[:, c0:c0 + cn],
                                xt_t[:, i, t * P:(t + 1) * P],
                                wv_t[:, i, c0:c0 + cn],
                                start=(i == 0), stop=(i == NDT - 1))
                    src = ps[:, 0:D].rearrange("p (h e) -> p h e", e=HD)
                    if vaug_t is not None:
                        dstv = vaug_t[:, t, :].rearrange(
                            "p (h e) -> p h e", e=AUG)[:, :, 0:HD]
                        nc.scalar.copy(dstv, src)
                    else:
                        ev = pool_ev.tile([P, H, HD], pdt, tag="evv")
                        nc.scalar.copy(ev[:], src)
                        nc.sync.dma_start(out=dst_dram[t], in_=ev[:])

            xt_t = pool_x.tile([P, NDT, N], ddt, tag="xt")
            nc.sync.dma_start(out=xt_t,
                              in_=xt_d[:].rearrange("(i p) n -> p i n", p=P))
            wv_t = pool_wv.tile([P, NDT, D], ddt, tag="wv")
            nc.sync.dma_start(out=wv_t,
                              in_=wv_d[:].rearrange("(i p) d -> p i d", p=P))

            qkv_T_form(xt_t, 0, qT, None)            # qT
            qkv_T_form(xt_t, D, kT, None)            # kT
            load_ones(vaug)
            v_form(xt_t, wv_t, vaug, None)           # v -> vaug

            x2t_t = pool_x.tile([P, NDT, N], ddt, tag="xt")
            nc.sync.dma_start(out=x2t_t,
                              in_=x2t_d[:].rearrange("(i p) n -> p i n", p=P))
            qkv_T_form(x2t_t, D, None, k2_spill)     # k2T -> dram
            v_form(x2t_t, wv_t, None, v2_spill)      # v2 -> dram

        # ================= phase B: attention + proj ====================
        pool_pt = top.enter_context(tc.tile_pool(name="pt", bufs=2))
        pool_ot = top.enter_context(tc.tile_pool(name="ot", bufs=2))
        pool_res = top.enter_context(tc.tile_pool(name="res", bufs=2))
        pool_sm = top.enter_context(tc.tile_pool(name="sm", bufs=2))

        def proj_qi(ot_t, br, qi):
            ps = pp_o.tile([P, D], dt.float32, tag="O")
            for g in range(NDT):
                for c0, cn in ((0, 512), (512, 256)):
                    nc.tensor.matmul(
                        ps[:, c0:c0 + cn],
                        ot_t[:, g, qi * P:(qi + 1) * P],
                        wp_t[:, g, c0:c0 + cn],
                        start=(g == 0), stop=(g == NDT - 1))
            res = pool_res.tile([P, D], dt.float32, tag="res")
            nc.vector.tensor_add(res[:], ps[:], bias_t[:])
            nc.sync.dma_start(out=out_d[br, qi * P:(qi + 1) * P, :],
                              in_=res[:])

        def attention(kT_t, vaug_t, br, extra):
            """head-pair ladder: S (TensorE) -> exp (ScalarE) -> AV
            (TensorE), half-g pt granularity for pipeline decoupling."""
            HQ = NQT // 2
            ot = pool_ot.tile([P, NDT, N], ddt, tag="ot")
            for g in range(NDT):
                po = [pp_o.tile([AUG, N], dt.float32, tag="O",
                                name=f"po{br}_{g}_{hh}")
                      for hh in range(2)]

                def emit_av(pth, kj, kjl):
                    for hh in range(2):
                        h = 2 * g + hh
                        for c in range(2):
                            nc.tensor.matmul(
                                po[hh][:, c * 512:(c + 1) * 512],
                                vaug_t[:, kj, h * AUG:(h + 1) * AUG],
                                pth[:, hh, kjl, c * 512:(c + 1) * 512],
                                start=(kj == 0), stop=(kj == NQT - 1),
                                skip_group_check=True)

                for half in range(2):
                    pth = pool_pt.tile([P, 2, HQ, N], pdt, tag="pt")
                    for kjl in range(HQ):
                        kj = half * HQ + kjl
                        pse = pp_s.tile([P, N], dt.float32, tag="S")
                        pso = pp_s.tile([P, N], dt.float32, tag="S")
                        for c in range(2):
                            nc.tensor.matmul(
                                pse[:, c * 512:(c + 1) * 512],
                                kT_t[0:HD, g, kj * P:(kj + 1) * P],
                                qT[0:HD, g, c * 512:(c + 1) * 512],
                                start=True, stop=True)
                            nc.tensor.matmul(
                                pso[:, c * 512:(c + 1) * 512],
                                kT_t[HD:P, g, kj * P:(kj + 1) * P],
                                qT[HD:P, g, c * 512:(c + 1) * 512],
                                start=True, stop=True)
                        nc.scalar.activation(
                            pth[:, 0, kjl, :], pse[:],
                            mybir.ActivationFunctionType.Exp, scale=SCALE)
                        nc.scalar.activation(
                            pth[:, 1, kjl, :], pso[:],
                            mybir.ActivationFunctionType.Exp, scale=SCALE)
                    for kjl in range(HQ):
                        emit_av(pth, half * HQ + kjl, kjl)

                # normalize o^T by 1/rowsum.  The ones-rows live on one
                # partition each; DVE reciprocal is per-lane-throughput
                # bound, so bounce them to DRAM and read back as [32,64]
                # (64 elems/lane) before the reciprocal, then bounce the
                # result out again for the partition-broadcast read.
                rb1 = dram_rb.tile([2, N], dt.float32, tag="rb1")
                for hh in range(2):
                    rf = pool_sm.tile([1, N], dt.float32, tag="rf")
                    nc.vector.tensor_copy(rf[:], po[hh][HD:HD + 1, :])
                    nc.sync.dma_start(out=rb1[hh:hh + 1, :], in_=rf[:])
                rgs = pool_sm.tile([32, HD], dt.float32, tag="rgs")
                nc.sync.dma_start(
                    out=rgs[:],
                    in_=rb1[:].rearrange("h (p e) -> (h p) e", e=HD))
                rr = pool_sm.tile([32, HD], dt.float32, tag="rr")
                nc.vector.reciprocal(rr[:], rgs[:])
                rb2 = dram_rb.tile([2, N], dt.float32, tag="rb2")
                nc.sync.dma_start(
                    out=rb2[:].rearrange("h (p e) -> (h p) e", e=HD),
                    in_=rr[:])
                for hh in range(2):
                    rb = pool_sm.tile([HD, N], dt.float32, tag="rb")
                    nc.sync.dma_start(
                        out=rb[:],
                        in_=rb2[hh, :].partition_broadcast(HD))
                    nc.vector.tensor_mul(
                        ot[hh * HD:(hh + 1) * HD, g, :],
                        po[hh][0:HD, :], rb[:])
                if extra:
                    extra.pop(0)()
                    if len(extra) > NDT - g - 1:
                        extra.pop(0)()
            while extra:
                extra.pop(0)()
            return ot

        ot0 = attention(kT, vaug, 0, [])

        # branch 2 k/v reload, then attention with proj(br0) interleaved
        kT2 = pool_kv.tile([P, NDT, N], ddt, tag="kT")
        nc.sync.dma_start(out=kT2,
                          in_=k2_spill[:].rearrange("g p n -> p g n"))
        vaug2 = pool_kv.tile([P, NQT, H * AUG], pdt, tag="vaug")
        load_ones(vaug2)
        for t in range(NQT):
            nc.sync.dma_start(
                out=vaug2[:, t, :].rearrange("p (h e) -> p h e",
                                             e=AUG)[:, :, 0:HD],
                in_=v2_spill[t])

        extra = [(lambda qi=qi: proj_qi(ot0, 0, qi)) for qi in range(NQT)]
        ot1 = attention(kT2, vaug2, 1, extra)
        for qi in range(NQT):
            proj_qi(ot1, 1, qi)

    n = _split_multiwaits(nc)
    _built = (nc, n)
    return _built


def _host_prep(x, x2, qkv_w, proj_w, proj_b):
    """-> list of 8 per-core input maps. Matmul operands are float32
    (device treats them as float32r); P/V-side constants are bfloat16."""
    import ml_dtypes
    f32 = lambda a: np.ascontiguousarray(a, dtype=np.float32)

    xt = np.ascontiguousarray(np.transpose(np.asarray(x), (0, 2, 1)))
    x2t = np.ascontiguousarray(np.transpose(np.asarray(x2), (0, 2, 1)))
    wqk = f32(np.asarray(qkv_w)[:2 * D].T)      # [768, 1536]
    wv = f32(np.asarray(qkv_w)[2 * D:].T)       # [768, 768]
    wp = f32(np.asarray(proj_w).T)              # [768, 768]
    bias = np.broadcast_to(np.asarray(proj_b, dtype=np.float32),
                           (P, D)).copy()
    ones = np.ones((P, H, 1), dtype=ml_dtypes.bfloat16)
    maps = []
    for c in range(B):
        maps.append({
            "xt": f32(xt[c]), "x2t": f32(x2t[c]),
            "wqk": wqk, "wv": wv, "wp": wp, "bias": bias,
            "ones": ones,
        })
    return maps


def kernel(x, x2, qkv_w, proj_w, proj_b, trace=False, tmpdir=None):
    nc, _ = _build()
    from concourse.bass_utils import run_bass_kernel_spmd
    in_maps = _host_prep(x, x2, qkv_w, proj_w, proj_b)
    res = run_bass_kernel_spmd(nc, in_maps, list(range(B)), trace=trace,
                               tmpdir=tmpdir)
    kernel.last_exec_time_ns = res.exec_time_ns
    out = np.stack([res.results[c]["out"] for c in range(B)])  # [B,2,N,D]
    out1 = np.ascontiguousarray(out[:, 0])
    out2 = np.ascontiguousarray(out[:, 1])
    return (out1, out2)


kernel.last_exec_time_ns = None



# revision 14
# speedup vs baseline: 1.2640x; 1.1390x over previous
"""Two-branch attention kernel for Trainium2 (8 NeuronCores, batch-parallel).

out1 = proj(softmax(q k^T / 8) v),  out2 = proj(softmax(q k2^T / 8) v2)
with q,k,v from x and k2,v2 from x2 (q shared across branches).

Sharding: batch dim (8) -> one batch element per core. No collectives.

Layout strategy (per core, transpose-free attention, all-bf16 matmuls):
  host passes x^T, x2^T, W_qkv^T, W_proj^T (bf16), broadcast bias (f32).
  QKV(x): qT,kT [dim,tok] via W-stationary matmuls; v [tok,dim] via
        x-stationary matmuls into a ones-augmented buffer.
  QKV(x2): k2T/v2 formed as filler thunks interleaved into branch-0
        attention (keeps TensorE saturated so the HAM clock-gate stays
        at 2.4GHz); evictions on VectorE (ScalarE is exp-bound there).
  Attn: S^T tile = kT(stationary) @ qT(moving); exp on ScalarE (scale=1/8,
        no max subtraction -- scores are ~N(0,1), exp range is safe) writes
        P^T straight to SBUF; AV accumulates o_aug^T = [v|1]^T @ P^T giving
        both o^T and the softmax denominators r; normalize o^T by 1/r:
        bounce r rows via DRAM to a [32,64] layout (reciprocal is
        per-lane-bound), reciprocal once, bounce out, partition-broadcast
        read, multiply straight out of PSUM.
  Proj: o^T tiles stationary, stream W_proj^T, add bias, DMA out;
        branch-0 proj thunks interleaved into branch-1 attention.
"""
import sys
for _p in ('/opt/trn_rl_repo',):
    if _p not in sys.path:
        sys.path.insert(0, _p)

import numpy as np

# ----------------------------------------------------------------------------
MODE = 'bf16+fastnorm+fused-x2'  # informational only

B, N, D, H, HD = 8, 1024, 768, 12, 64
SCALE = HD ** -0.5
NDT = D // 128       # 6 dim tiles
NQT = N // 128       # 8 token tiles
P = 128

# ----------------------------------------------------------------------------
# workaround: walrus rejects >2 sem waits on one instruction; TileContext's
# tail drain carries one wait per active logical proc. Split them across
# single-wait SP nops and emit a bare drain.
def _install_tilefix():
    import bass_rust
    import concourse.tile as tile

    def _drain_and_barrier_split(self, tick_clock, wait_clock):
        gc = tick_clock.global_clock
        ticks = [gc[i] for i in range(27)]
        for i, t in enumerate(ticks):
            if t > 0:
                vc = bass_rust.VectorClock(
                    [t if j == i else 0 for j in range(len(ticks))])
                nop = self.nc.sync.nop()
                wait_clock.add_sem_waits(
                    nop.ins, bass_rust.ScopedClock({None: vc}))
        self.nc.sync.drain()
        self.nc.all_engine_barrier()
        assert self.sems is not None
        popped = self.nc._tile_sem_poison_stack.pop()
        assert popped is self._sem_poison
        self.nc.clear_and_free_semaphores(list(self.sems.allocated().values()))
        self.nc.all_engine_barrier()

    tile.TileContext._drain_and_barrier = _drain_and_barrier_split


def _split_multiwaits(nc, max_waits=1):
    """walrus codegen rejects instructions carrying more than `max_waits`
    sync waits; hoist the extras onto same-engine nops placed just before."""
    import bass_rust
    import concourse.mybir as mybir
    cnt = 0
    for bb in nc.main_func.blocks:
        insts = bb.instructions
        i = 0
        while i < len(insts):
            ins = insts[i]
            si = getattr(ins, 'sync_info', None)
            if si is not None and si.on_wait and len(si.on_wait) > max_waits:
                waits = list(si.on_wait)
                extras, keep = waits[:-max_waits], waits[-max_waits:]
                for w in extras:
                    nop = mybir.InstNoOp(name=f"I-swx{cnt}", ins=[], outs=[])
                    cnt += 1
                    nop.engine = ins.engine
                    nop.sync_info = bass_rust.SyncInfo(on_wait=[w],
                                                       on_update=[])
                    insts.insert(i, nop)
                    i += 1
                ins.sync_info = bass_rust.SyncInfo(
                    on_wait=keep, on_update=list(si.on_update))
            i += 1
    return cnt


_built = None


def _build():
    """Build the SPMD bass program once. Returns (nc, n_split_waits)."""
    global _built
    if _built is not None:
        return _built
    _install_tilefix()
    from contextlib import ExitStack
    import concourse.bass as bass
    import concourse.tile as tile
    from concourse import mybir

    dt = mybir.dt
    bdt = dt.bfloat16          # matmul operand dtype throughout

    nc = bass.Bass("TRN2", target_bir_lowering=False, debug=False,
                   num_devices=8)

    # DRAM I/O (per core)
    xt_d = nc.dram_tensor("xt", [D, N], bdt, kind="ExternalInput")
    x2t_d = nc.dram_tensor("x2t", [D, N], bdt, kind="ExternalInput")
    wqk_d = nc.dram_tensor("wqk", [D, 2 * D], bdt, kind="ExternalInput")
    wv_d = nc.dram_tensor("wv", [D, D], bdt, kind="ExternalInput")
    wp_d = nc.dram_tensor("wp", [D, D], bdt, kind="ExternalInput")
    bias_d = nc.dram_tensor("bias", [P, D], dt.float32, kind="ExternalInput")
    ones_d = nc.dram_tensor("ones", [P, H, 1], bdt, kind="ExternalInput")
    out_d = nc.dram_tensor("out", [2, N, D], dt.float32,
                           kind="ExternalOutput")

    AUG = HD + 1  # 65: head dim + ones column for row sums

    with tile.TileContext(nc) as tc, ExitStack() as top:
        pp_s = top.enter_context(tc.tile_pool(name="ps_s", bufs=2,
                                              space="PSUM"))
        pp_o = top.enter_context(tc.tile_pool(name="ps_o", bufs=2,
                                              space="PSUM"))
        dram_rb = top.enter_context(tc.tile_pool(name="dram_rb", bufs=2,
                                                 space="DRAM"))
        persist = top.enter_context(tc.tile_pool(name="persist", bufs=1))
        pool_kv = top.enter_context(tc.tile_pool(name="kv", bufs=1))
        pool_wst = top.enter_context(tc.tile_pool(name="wst", bufs=10))

        qT = persist.tile([P, NDT, N], bdt, tag="qT")
        wp_t = persist.tile([P, NDT, D], bdt, tag="wp")
        nc.sync.dma_start(
            out=wp_t, in_=wp_d[:].rearrange("(g p) d -> p g d", p=P))
        bias_t = persist.tile([P, D], dt.float32, tag="bias")
        nc.sync.dma_start(out=bias_t, in_=bias_d[:])

        # k/v for both branches + x2^T + wv stay resident through phase B
        kT = pool_kv.tile([P, NDT, N], bdt, tag="kT")
        kT2 = pool_kv.tile([P, NDT, N], bdt, tag="kT2")
        vaug = pool_kv.tile([P, NQT, H * AUG], bdt, tag="vaug")
        vaug2 = pool_kv.tile([P, NQT, H * AUG], bdt, tag="vaug2")
        x2t_t = pool_kv.tile([P, NDT, N], bdt, tag="x2t")
        wv_t = pool_kv.tile([P, NDT, D], bdt, tag="wv")

        def load_ones(vaug_t):
            for t in range(NQT):
                nc.sync.dma_start(
                    out=vaug_t[:, t, :].rearrange("p (h e) -> p h e",
                                                  e=AUG)[:, :, HD:AUG],
                    in_=ones_d[:])

        def qkv_T_group(xt_tile, wcol0, o, dst_sb, evict):
            """one [128, N] output group accumulated over 6 input tiles."""
            ps = pp_s.tile([P, N], dt.float32, tag="S")
            for i in range(NDT):
                wt = pool_wst.tile([P, P], bdt, tag="wt")
                nc.sync.dma_start(
                    out=wt,
                    in_=wqk_d[i * P:(i + 1) * P,
                              wcol0 + o * P: wcol0 + (o + 1) * P])
                for c in range(2):
                    nc.tensor.matmul(
                        ps[:, c * 512:(c + 1) * 512],
                        wt[:],
                        xt_tile[:, i, c * 512:(c + 1) * 512],
                        start=(i == 0), stop=(i == NDT - 1))
            evict(dst_sb[:, o, :], ps[:])

        def v_tile(xt_tile, vaug_t, t, evict):
            """one [128 tok, 768] v tile accumulated over 6 input tiles."""
            ps = pp_s.tile([P, N], dt.float32, tag="S")
            for i in range(NDT):
                for c0, cn in ((0, 512), (512, 256)):
                    nc.tensor.matmul(
                        ps[:, c0:c0 + cn],
                        xt_tile[:, i, t * P:(t + 1) * P],
                        wv_t[:, i, c0:c0 + cn],
                        start=(i == 0), stop=(i == NDT - 1))
            src = ps[:, 0:D].rearrange("p (h e) -> p h e", e=HD)
            dstv = vaug_t[:, t, :].rearrange("p (h e) -> p h e",
                                             e=AUG)[:, :, 0:HD]
            evict(dstv, src)

        # ================= phase A: QKV for x ===========================
        with tc.tile_pool(name="xa", bufs=1) as pool_x:
            xt_t = pool_x.tile([P, NDT, N], bdt, tag="xt")
            nc.sync.dma_start(out=xt_t,
                              in_=xt_d[:].rearrange("(i p) n -> p i n", p=P))
            nc.sync.dma_start(out=wv_t,
                              in_=wv_d[:].rearrange("(i p) d -> p i d", p=P))
            nc.sync.dma_start(out=x2t_t,
                              in_=x2t_d[:].rearrange("(i p) n -> p i n",
                                                     p=P))
            for o in range(NDT):
                qkv_T_group(xt_t, 0, o, qT, nc.scalar.copy)
            for o in range(NDT):
                qkv_T_group(xt_t, D, o, kT, nc.scalar.copy)
            load_ones(vaug)
            load_ones(vaug2)
            for t in range(NQT):
                v_tile(xt_t, vaug, t, nc.scalar.copy)

        # ================= phase B: attention + proj ====================
        pool_pt = top.enter_context(tc.tile_pool(name="pt", bufs=2))
        pool_ot = top.enter_context(tc.tile_pool(name="ot", bufs=2))
        pool_res = top.enter_context(tc.tile_pool(name="res", bufs=2))
        pool_sm = top.enter_context(tc.tile_pool(name="sm", bufs=2))

        def proj_qi(ot_t, br, qi):
            ps = pp_o.tile([P, D], dt.float32, tag="O")
            for g in range(NDT):
                for c0, cn in ((0, 512), (512, 256)):
                    nc.tensor.matmul(
                        ps[:, c0:c0 + cn],
                        ot_t[:, g, qi * P:(qi + 1) * P],
                        wp_t[:, g, c0:c0 + cn],
                        start=(g == 0), stop=(g == NDT - 1))
            res = pool_res.tile([P, D], dt.float32, tag="res")
            nc.vector.tensor_add(res[:], ps[:], bias_t[:])
            nc.sync.dma_start(out=out_d[br, qi * P:(qi + 1) * P, :],
                              in_=res[:])

        def attention(kT_t, vaug_t, br, extra):
            """S (TensorE) -> exp (ScalarE) -> AV (TensorE), half-g pt
            granularity; `extra` thunks are popped per-g to keep TensorE
            saturated (HAM clock-gate stays warm)."""
            HQ = NQT // 2
            n0 = len(extra)
            ot = pool_ot.tile([P, NDT, N], bdt, tag="ot")
            for g in range(NDT):
                po = [pp_o.tile([AUG, N], dt.float32, tag="O",
                                name=f"po{br}_{g}_{hh}")
                      for hh in range(2)]

                def emit_av(pth, kj, kjl):
                    for hh in range(2):
                        h = 2 * g + hh
                        for c in range(2):
                            nc.tensor.matmul(
                                po[hh][:, c * 512:(c + 1) * 512],
                                vaug_t[:, kj, h * AUG:(h + 1) * AUG],
                                pth[:, hh, kjl, c * 512:(c + 1) * 512],
                                start=(kj == 0), stop=(kj == NQT - 1),
                                skip_group_check=True)

                for half in range(2):
                    pth = pool_pt.tile([P, 2, HQ, N], bdt, tag="pt")
                    for kjl in range(HQ):
                        kj = half * HQ + kjl
                        pse = pp_s.tile([P, N], dt.float32, tag="S")
                        pso = pp_s.tile([P, N], dt.float32, tag="S")
                        for c in range(2):
                            nc.tensor.matmul(
                                pse[:, c * 512:(c + 1) * 512],
                                kT_t[0:HD, g, kj * P:(kj + 1) * P],
                                qT[0:HD, g, c * 512:(c + 1) * 512],
                                start=True, stop=True)
                            nc.tensor.matmul(
                                pso[:, c * 512:(c + 1) * 512],
                                kT_t[HD:P, g, kj * P:(kj + 1) * P],
                                qT[HD:P, g, c * 512:(c + 1) * 512],
                                start=True, stop=True)
                        nc.scalar.activation(
                            pth[:, 0, kjl, :], pse[:],
                            mybir.ActivationFunctionType.Exp, scale=SCALE)
                        nc.scalar.activation(
                            pth[:, 1, kjl, :], pso[:],
                            mybir.ActivationFunctionType.Exp, scale=SCALE)
                    for kjl in range(HQ):
                        emit_av(pth, half * HQ + kjl, kjl)

                # normalize o^T by 1/rowsum (see module docstring)
                rb1 = dram_rb.tile([2, N], dt.float32, tag="rb1")
                for hh in range(2):
                    rf = pool_sm.tile([1, N], dt.float32, tag="rf")
                    nc.vector.tensor_copy(rf[:], po[hh][HD:HD + 1, :])
                    nc.sync.dma_start(out=rb1[hh:hh + 1, :], in_=rf[:])
                rgs = pool_sm.tile([32, HD], dt.float32, tag="rgs")
                nc.sync.dma_start(
                    out=rgs[:],
                    in_=rb1[:].rearrange("h (p e) -> (h p) e", e=HD))
                rr = pool_sm.tile([32, HD], dt.float32, tag="rr")
                nc.vector.reciprocal(rr[:], rgs[:])
                rb2 = dram_rb.tile([2, N], dt.float32, tag="rb2")
                nc.sync.dma_start(
                    out=rb2[:].rearrange("h (p e) -> (h p) e", e=HD),
                    in_=rr[:])
                for hh in range(2):
                    rb = pool_sm.tile([HD, N], dt.float32, tag="rb")
                    nc.sync.dma_start(
                        out=rb[:],
                        in_=rb2[hh, :].partition_broadcast(HD))
                    nc.vector.tensor_mul(
                        ot[hh * HD:(hh + 1) * HD, g, :],
                        po[hh][0:HD, :], rb[:])

                # pace filler thunks: leave n0*(NDT-1-g)/NDT for later g's
                want_left = n0 * (NDT - 1 - g) // NDT
                while len(extra) > want_left:
                    extra.pop(0)()
            while extra:
                extra.pop(0)()
            return ot

        # branch 0 attention with x2 k2/v2 formation as filler.
        # evictions on VectorE (ScalarE is exp-bound during attention).
        ev_vec = nc.vector.tensor_copy
        thunks = [(lambda o=o: qkv_T_group(x2t_t, D, o, kT2, ev_vec))
                  for o in range(NDT)]
        thunks += [(lambda t=t: v_tile(x2t_t, vaug2, t, ev_vec))
                   for t in range(NQT)]
        ot0 = attention(kT, vaug, 0, thunks)

        # branch 1 attention with branch-0 proj as filler
        extra = [(lambda qi=qi: proj_qi(ot0, 0, qi)) for qi in range(NQT)]
        ot1 = attention(kT2, vaug2, 1, extra)
        for qi in range(NQT):
            proj_qi(ot1, 1, qi)

    n = _split_multiwaits(nc)
    _built = (nc, n)
    return _built


def _host_prep(x, x2, qkv_w, proj_w, proj_b):
    """-> list of 8 per-core input maps; matmul operands in bfloat16."""
    import ml_dtypes
    bf16 = ml_dtypes.bfloat16
    b16 = lambda a: np.ascontiguousarray(np.asarray(a), dtype=bf16)

    xt = b16(np.transpose(np.asarray(x), (0, 2, 1)))
    x2t = b16(np.transpose(np.asarray(x2), (0, 2, 1)))
    wqk = b16(np.asarray(qkv_w)[:2 * D].T)      # [768, 1536]
    wv = b16(np.asarray(qkv_w)[2 * D:].T)       # [768, 768]
    wp = b16(np.asarray(proj_w).T)              # [768, 768]
    bias = np.broadcast_to(np.asarray(proj_b, dtype=np.float32),
                           (P, D)).copy()
    ones = np.ones((P, H, 1), dtype=bf16)
    maps = []
    for c in range(B):
        maps.append({
            "xt": np.ascontiguousarray(xt[c]),
            "x2t": np.ascontiguousarray(x2t[c]),
            "wqk": wqk, "wv": wv, "wp": wp, "bias": bias,
            "ones": ones,
        })
    return maps


def kernel(x, x2, qkv_w, proj_w, proj_b, trace=False, tmpdir=None):
    nc, _ = _build()
    from concourse.bass_utils import run_bass_kernel_spmd
    in_maps = _host_prep(x, x2, qkv_w, proj_w, proj_b)
    res = run_bass_kernel_spmd(nc, in_maps, list(range(B)), trace=trace,
                               tmpdir=tmpdir)
    kernel.last_exec_time_ns = res.exec_time_ns
    out = np.stack([res.results[c]["out"] for c in range(B)])  # [B,2,N,D]
    out1 = np.ascontiguousarray(out[:, 0])
    out2 = np.ascontiguousarray(out[:, 1])
    return (out1, out2)


kernel.last_exec_time_ns = None


# revision 21
# speedup vs baseline: 1.2818x; 1.0140x over previous
"""Two-branch attention kernel for Trainium2 (8 NeuronCores, batch-parallel).

out1 = proj(softmax(q k^T / 8) v),  out2 = proj(softmax(q k2^T / 8) v2)
with q,k,v from x and k2,v2 from x2 (q shared across branches).

Sharding: batch dim (8) -> one batch element per core. No collectives.

Layout strategy (per core, transpose-free attention, all-bf16 matmuls):
  host passes x^T, x2^T, W_qkv^T, W_proj^T (bf16), broadcast bias (f32).
  QKV(x): qT,kT [dim,tok] via W-stationary matmuls; v [tok,dim] via
        x-stationary matmuls into a ones-augmented buffer.
  QKV(x2): k2T/v2 formed as filler thunks interleaved into branch-0
        attention (keeps TensorE saturated so the HAM clock-gate stays
        at 2.4GHz); evictions on VectorE (ScalarE is exp-bound there).
  Attn: S^T tile = kT(stationary) @ qT(moving); exp on ScalarE (scale=1/8,
        no max subtraction -- scores are ~N(0,1), exp range is safe) writes
        P^T straight to SBUF; AV accumulates o_aug^T = [v|1]^T @ P^T giving
        both o^T and the softmax denominators r; normalize o^T by 1/r:
        bounce r rows via DRAM to a [32,64] layout (reciprocal is
        per-lane-bound), reciprocal once, bounce out, partition-broadcast
        read, multiply straight out of PSUM.
  Proj: o^T tiles stationary, stream W_proj^T, add bias, DMA out;
        branch-0 proj thunks interleaved into branch-1 attention.
"""
import sys
for _p in ('/opt/trn_rl_repo',):
    if _p not in sys.path:
        sys.path.insert(0, _p)

import numpy as np

# ----------------------------------------------------------------------------
MODE = 'bf16+fastnorm+fused-x2'  # informational only

B, N, D, H, HD = 8, 1024, 768, 12, 64
SCALE = HD ** -0.5
NDT = D // 128       # 6 dim tiles
NQT = N // 128       # 8 token tiles
P = 128

# ----------------------------------------------------------------------------
# workaround: walrus rejects >2 sem waits on one instruction; TileContext's
# tail drain carries one wait per active logical proc. Split them across
# single-wait SP nops and emit a bare drain.
def _install_tilefix():
    import bass_rust
    import concourse.tile as tile

    def _drain_and_barrier_split(self, tick_clock, wait_clock):
        gc = tick_clock.global_clock
        ticks = [gc[i] for i in range(27)]
        for i, t in enumerate(ticks):
            if t > 0:
                vc = bass_rust.VectorClock(
                    [t if j == i else 0 for j in range(len(ticks))])
                nop = self.nc.sync.nop()
                wait_clock.add_sem_waits(
                    nop.ins, bass_rust.ScopedClock({None: vc}))
        self.nc.sync.drain()
        self.nc.all_engine_barrier()
        assert self.sems is not None
        popped = self.nc._tile_sem_poison_stack.pop()
        assert popped is self._sem_poison
        self.nc.clear_and_free_semaphores(list(self.sems.allocated().values()))
        self.nc.all_engine_barrier()

    tile.TileContext._drain_and_barrier = _drain_and_barrier_split


def _split_multiwaits(nc, max_waits=1):
    """walrus codegen rejects instructions carrying more than `max_waits`
    sync waits; hoist the extras onto same-engine nops placed just before."""
    import bass_rust
    import concourse.mybir as mybir
    cnt = 0
    for bb in nc.main_func.blocks:
        insts = bb.instructions
        i = 0
        while i < len(insts):
            ins = insts[i]
            si = getattr(ins, 'sync_info', None)
            if si is not None and si.on_wait and len(si.on_wait) > max_waits:
                waits = list(si.on_wait)
                extras, keep = waits[:-max_waits], waits[-max_waits:]
                for w in extras:
                    nop = mybir.InstNoOp(name=f"I-swx{cnt}", ins=[], outs=[])
                    cnt += 1
                    nop.engine = ins.engine
                    nop.sync_info = bass_rust.SyncInfo(on_wait=[w],
                                                       on_update=[])
                    insts.insert(i, nop)
                    i += 1
                ins.sync_info = bass_rust.SyncInfo(
                    on_wait=keep, on_update=list(si.on_update))
            i += 1
    return cnt


_built = None


def _build():
    """Build the SPMD bass program once. Returns (nc, n_split_waits)."""
    global _built
    if _built is not None:
        return _built
    _install_tilefix()
    from contextlib import ExitStack
    import concourse.bass as bass
    import concourse.tile as tile
    from concourse import mybir

    dt = mybir.dt
    bdt = dt.bfloat16          # matmul operand dtype throughout

    nc = bass.Bass("TRN2", target_bir_lowering=False, debug=False,
                   num_devices=8)

    # DRAM I/O (per core); x/w tensors come in p-major layout so the big
    # loads are 128 fat contiguous descriptors instead of 768 thin ones.
    xt_d = nc.dram_tensor("xt", [P, NDT, N], bdt, kind="ExternalInput")
    x2t_d = nc.dram_tensor("x2t", [P, NDT, N], bdt, kind="ExternalInput")
    wqk_d = nc.dram_tensor("wqk", [D, 2 * D], bdt, kind="ExternalInput")
    wv_d = nc.dram_tensor("wv", [P, NDT, D], bdt, kind="ExternalInput")
    wp_d = nc.dram_tensor("wp", [P, NDT, D], bdt, kind="ExternalInput")
    bias_d = nc.dram_tensor("bias", [P, D], dt.float32, kind="ExternalInput")
    ones_d = nc.dram_tensor("ones", [P, H, 1], bdt, kind="ExternalInput")
    out_d = nc.dram_tensor("out", [2, N, D], dt.float32,
                           kind="ExternalOutput")

    AUG = HD + 1  # 65: head dim + ones column for row sums

    with tile.TileContext(nc) as tc, ExitStack() as top:
        pp_s = top.enter_context(tc.tile_pool(name="ps_s", bufs=2,
                                              space="PSUM"))
        pp_o = top.enter_context(tc.tile_pool(name="ps_o", bufs=2,
                                              space="PSUM"))
        dram_rb = top.enter_context(tc.tile_pool(name="dram_rb", bufs=2,
                                                 space="DRAM"))
        persist = top.enter_context(tc.tile_pool(name="persist", bufs=1))
        pool_kv = top.enter_context(tc.tile_pool(name="kv", bufs=1))
        pool_wst = top.enter_context(tc.tile_pool(name="wst", bufs=10))

        qT = persist.tile([P, NDT, N], bdt, tag="qT")
        wp_t = persist.tile([P, NDT, D], bdt, tag="wp")
        nc.sync.dma_start(out=wp_t, in_=wp_d[:])
        bias_t = persist.tile([P, D], dt.float32, tag="bias")
        nc.sync.dma_start(out=bias_t, in_=bias_d[:])

        # x/k/v for both branches + wv stay resident through phase B
        kT = pool_kv.tile([P, NDT, N], bdt, tag="kT")
        kT2 = pool_kv.tile([P, NDT, N], bdt, tag="kT2")
        vaug = pool_kv.tile([P, NQT, H * AUG], bdt, tag="vaug")
        vaug2 = pool_kv.tile([P, NQT, H * AUG], bdt, tag="vaug2")
        xt_t = pool_kv.tile([P, NDT, N], bdt, tag="xt")
        x2t_t = pool_kv.tile([P, NDT, N], bdt, tag="x2t")
        wv_t = pool_kv.tile([P, NDT, D], bdt, tag="wv")

        def load_ones(vaug_t):
            for t in range(NQT):
                nc.sync.dma_start(
                    out=vaug_t[:, t, :].rearrange("p (h e) -> p h e",
                                                  e=AUG)[:, :, HD:AUG],
                    in_=ones_d[:])

        def qkv_T_group(xt_tile, wcol0, o, dst_sb, evict):
            """one [128, N] output group accumulated over 6 input tiles."""
            ps = pp_s.tile([P, N], dt.float32, tag="S")
            for i in range(NDT):
                wt = pool_wst.tile([P, P], bdt, tag="wt")
                nc.sync.dma_start(
                    out=wt,
                    in_=wqk_d[i * P:(i + 1) * P,
                              wcol0 + o * P: wcol0 + (o + 1) * P])
                for c in range(2):
                    nc.tensor.matmul(
                        ps[:, c * 512:(c + 1) * 512],
                        wt[:],
                        xt_tile[:, i, c * 512:(c + 1) * 512],
                        start=(i == 0), stop=(i == NDT - 1))
            evict(dst_sb[:, o, :], ps[:])

        def v_tile(xt_tile, vaug_t, t, evict):
            """one [128 tok, 768] v tile accumulated over 6 input tiles."""
            ps = pp_s.tile([P, N], dt.float32, tag="S")
            for i in range(NDT):
                for c0, cn in ((0, 512), (512, 256)):
                    nc.tensor.matmul(
                        ps[:, c0:c0 + cn],
                        xt_tile[:, i, t * P:(t + 1) * P],
                        wv_t[:, i, c0:c0 + cn],
                        start=(i == 0), stop=(i == NDT - 1))
            src = ps[:, 0:D].rearrange("p (h e) -> p h e", e=HD)
            dstv = vaug_t[:, t, :].rearrange("p (h e) -> p h e",
                                             e=AUG)[:, :, 0:HD]
            evict(dstv, src)

        # ============ phase A prefix: just enough to start attention ====
        nc.sync.dma_start(out=xt_t, in_=xt_d[:])
        nc.sync.dma_start(out=wv_t, in_=wv_d[:])
        nc.sync.dma_start(out=x2t_t, in_=x2t_d[:])
        qkv_T_group(xt_t, 0, 0, qT, nc.scalar.copy)
        qkv_T_group(xt_t, D, 0, kT, nc.scalar.copy)
        load_ones(vaug)
        load_ones(vaug2)
        for t in range(NQT):
            v_tile(xt_t, vaug, t, nc.scalar.copy)

        # ================= phase B: attention + proj ====================
        pool_pt = top.enter_context(tc.tile_pool(name="pt", bufs=2))
        pool_ot = top.enter_context(tc.tile_pool(name="ot", bufs=2))
        pool_res = top.enter_context(tc.tile_pool(name="res", bufs=2))
        pool_sm = top.enter_context(tc.tile_pool(name="sm", bufs=2))

        def proj_qi(ot_t, br, qi):
            ps = pp_o.tile([P, D], dt.float32, tag="O")
            for g in range(NDT):
                for c0, cn in ((0, 512), (512, 256)):
                    nc.tensor.matmul(
                        ps[:, c0:c0 + cn],
                        ot_t[:, g, qi * P:(qi + 1) * P],
                        wp_t[:, g, c0:c0 + cn],
                        start=(g == 0), stop=(g == NDT - 1))
            res = pool_res.tile([P, D], dt.float32, tag="res")
            nc.vector.tensor_add(res[:], ps[:], bias_t[:])
            nc.sync.dma_start(out=out_d[br, qi * P:(qi + 1) * P, :],
                              in_=res[:])

        def attention(kT_t, vaug_t, br, extra, hold=0):
            """S (TensorE) -> exp (ScalarE) -> AV (TensorE), half-g pt
            granularity; `extra` thunks are popped per-half to keep
            TensorE saturated (HAM clock-gate stays warm).  `hold` thunks
            are left in `extra` for the caller's tail."""
            HQ = NQT // 2
            n0 = len(extra) - hold
            nhalf = 2 * NDT
            ot = pool_ot.tile([P, NDT, N], bdt, tag="ot")
            for g in range(NDT):
                po = [pp_o.tile([AUG, N], dt.float32, tag="O",
                                name=f"po{br}_{g}_{hh}")
                      for hh in range(2)]

                def emit_av(pth, kj, kjl):
                    for hh in range(2):
                        h = 2 * g + hh
                        for c in range(2):
                            nc.tensor.matmul(
                                po[hh][:, c * 512:(c + 1) * 512],
                                vaug_t[:, kj, h * AUG:(h + 1) * AUG],
                                pth[:, hh, kjl, c * 512:(c + 1) * 512],
                                start=(kj == 0), stop=(kj == NQT - 1),
                                skip_group_check=True)

                for half in range(2):
                    pth = pool_pt.tile([P, 2, HQ, N], bdt, tag="pt")
                    for kjl in range(HQ):
                        kj = half * HQ + kjl
                        pse = pp_s.tile([P, N], dt.float32, tag="S")
                        pso = pp_s.tile([P, N], dt.float32, tag="S")
                        for c in range(2):
                            nc.tensor.matmul(
                                pse[:, c * 512:(c + 1) * 512],
                                kT_t[0:HD, g, kj * P:(kj + 1) * P],
                                qT[0:HD, g, c * 512:(c + 1) * 512],
                                start=True, stop=True)
                            nc.tensor.matmul(
                                pso[:, c * 512:(c + 1) * 512],
                                kT_t[HD:P, g, kj * P:(kj + 1) * P],
                                qT[HD:P, g, c * 512:(c + 1) * 512],
                                start=True, stop=True)
                        nc.scalar.activation(
                            pth[:, 0, kjl, :], pse[:],
                            mybir.ActivationFunctionType.Exp, scale=SCALE)
                        nc.scalar.activation(
                            pth[:, 1, kjl, :], pso[:],
                            mybir.ActivationFunctionType.Exp, scale=SCALE)
                    for kjl in range(HQ):
                        emit_av(pth, half * HQ + kjl, kjl)
                    hidx = 2 * g + half
                    want_left = hold + n0 * (nhalf - 1 - hidx) // nhalf
                    while len(extra) > want_left:
                        extra.pop(0)()

                # normalize o^T by 1/rowsum (see module docstring)
                rb1 = dram_rb.tile([2, N], dt.float32, tag="rb1")
                for hh in range(2):
                    rf = pool_sm.tile([1, N], dt.float32, tag="rf")
                    nc.vector.tensor_copy(rf[:], po[hh][HD:HD + 1, :])
                    nc.sync.dma_start(out=rb1[hh:hh + 1, :], in_=rf[:])
                rgs = pool_sm.tile([32, HD], dt.float32, tag="rgs")
                nc.sync.dma_start(
                    out=rgs[:],
                    in_=rb1[:].rearrange("h (p e) -> (h p) e", e=HD))
                rr = pool_sm.tile([32, HD], dt.float32, tag="rr")
                nc.vector.reciprocal(rr[:], rgs[:])
                rb2 = dram_rb.tile([2, N], dt.float32, tag="rb2")
                nc.sync.dma_start(
                    out=rb2[:].rearrange("h (p e) -> (h p) e", e=HD),
                    in_=rr[:])
                for hh in range(2):
                    rb = pool_sm.tile([HD, N], dt.float32, tag="rb")
                    nc.sync.dma_start(
                        out=rb[:],
                        in_=rb2[hh, :].partition_broadcast(HD))
                    nc.vector.tensor_mul(
                        ot[hh * HD:(hh + 1) * HD, g, :],
                        po[hh][0:HD, :], rb[:])

            while len(extra) > hold:
                extra.pop(0)()
            return ot

        # branch 0 attention with the rest of QKV as filler: qT/kT groups
        # 1-5 (needed one g ahead), then x2's k2T groups and v2 tiles.
        # Evictions on VectorE (ScalarE is exp-bound during attention).
        ev_vec = nc.vector.tensor_copy
        thunks = []
        for o in range(1, NDT):
            thunks.append(lambda o=o: qkv_T_group(xt_t, 0, o, qT, ev_vec))
            thunks.append(lambda o=o: qkv_T_group(xt_t, D, o, kT, ev_vec))
        thunks += [(lambda o=o: qkv_T_group(x2t_t, D, o, kT2, ev_vec))
                   for o in range(NDT)]
        thunks += [(lambda t=t: v_tile(x2t_t, vaug2, t, ev_vec))
                   for t in range(NQT)]
        ot0 = attention(kT, vaug, 0, thunks)

        # branch 1 attention with branch-0 proj as filler; hold a few back
        # to cover the normalize-chain latency of the last g before the
        # branch-1 projs can start.
        extra = [(lambda qi=qi: proj_qi(ot0, 0, qi)) for qi in range(NQT)]
        ot1 = attention(kT2, vaug2, 1, extra, hold=3)
        for qi in range(NQT):
            proj_qi(ot1, 1, qi)
            if extra:
                extra.pop(0)()

    n = _split_multiwaits(nc)
    _built = (nc, n)
    return _built


def _host_prep(x, x2, qkv_w, proj_w, proj_b):
    """-> list of 8 per-core input maps; matmul operands in bfloat16.
    x^T/wv/wp are sent p-major ([128, i, cols], partition-dim first) so
    the device-side loads are fat contiguous descriptors."""
    import ml_dtypes
    bf16 = ml_dtypes.bfloat16
    b16 = lambda a: np.ascontiguousarray(np.asarray(a), dtype=bf16)

    def pmaj(m):  # [768, cols] -> [128, 6, cols]
        return np.ascontiguousarray(
            np.asarray(m).reshape(NDT, P, -1).transpose(1, 0, 2))

    xt = b16(np.transpose(np.asarray(x), (0, 2, 1)))     # [B, 768, 1024]
    x2t = b16(np.transpose(np.asarray(x2), (0, 2, 1)))
    wqk = b16(np.asarray(qkv_w)[:2 * D].T)               # [768, 1536]
    wv = b16(pmaj(np.asarray(qkv_w)[2 * D:].T))          # [128, 6, 768]
    wp = b16(pmaj(np.asarray(proj_w).T))                 # [128, 6, 768]
    bias = np.broadcast_to(np.asarray(proj_b, dtype=np.float32),
                           (P, D)).copy()
    ones = np.ones((P, H, 1), dtype=bf16)
    maps = []
    for c in range(B):
        maps.append({
            "xt": b16(pmaj(xt[c])),
            "x2t": b16(pmaj(x2t[c])),
            "wqk": wqk, "wv": wv, "wp": wp, "bias": bias,
            "ones": ones,
        })
    return maps


def kernel(x, x2, qkv_w, proj_w, proj_b, trace=False, tmpdir=None):
    nc, _ = _build()
    from concourse.bass_utils import run_bass_kernel_spmd
    in_maps = _host_prep(x, x2, qkv_w, proj_w, proj_b)
    res = run_bass_kernel_spmd(nc, in_maps, list(range(B)), trace=trace,
                               tmpdir=tmpdir)
    kernel.last_exec_time_ns = res.exec_time_ns
    out = np.stack([res.results[c]["out"] for c in range(B)])  # [B,2,N,D]
    out1 = np.ascontiguousarray(out[:, 0])
    out2 = np.ascontiguousarray(out[:, 1])
    return (out1, out2)


kernel.last_exec_time_ns = None
